# revision 1
# baseline (speedup 1.0000x reference)
"""Single-head causal attention (prefill) on 8 Trainium2 NeuronCores.

Problem: x[4,4096,2048], Wq/Wk/Wv[2048,128] -> out[4,4096,128]
  Q=xWq K=xWk V=xWv; out = softmax(mask(QK^T/sqrt(128))) V

Sharding: data-parallel over batch (4) x 2-way balanced causal query split.
Core c handles batch c//2; half h=c%2 takes query chunks (of 512)
  h=0: {0,3,4,7}   h=1: {1,2,5,6}
which balances causal attention FLOPs. A single SPMD program serves both
halves: slot s processes 512 queries against the first NK[s] key columns
(NK=[1024,2048,3072,4096]); the host permutes the key columns of its
pre-transposed x (h=1 swaps adjacent 512-blocks) so each core's needed keys
always form a prefix, and per-core causal masks are threshold DATA
(thr[s,i,k]: query column threshold per key row) applied on-device as
  e = (iota >= thr) * exp(scores)

On-device dataflow (per core), all matmuls fp32r (2-pass fp32 on the PE,
~2x faster than plain fp32; measured ~2.4e-4 max rel err end to end):
  xT tiles [e=128,512] stream from DRAM (host supplies x pre-transposed)
  KT/VT/QT[d,t] = W[e,d]-chunk^T @ xT       (accumulate 16 e-chunks in PSUM)
  V[t,d]       = PE-transpose of VT 128-blocks
  scoresT[k,q] = KT-chunk^T... lhsT=KT[:,kc] rhs=QT[:,slot]   (one matmul)
  e[k,q]       = ACT exp(scale*scoresT) (PSUM->SBUF), masked via DVE STT
  outT[d,q]   += lhsT=V-chunk rhs=e          (accumulate over k-chunks)
  den[1,q]    += lhsT=ones[k,1] rhs=(e_c + e_c+1)  (DVE pair-sum halves the
                                                    denominator matmuls)
  out          = outT * broadcast(approx_recip(den)) -> DRAM [slot,d,q]
Host reassembles out[b,t,d] from per-core [4,128,512] slot outputs.

Measured on trn2 (axon): ~173 us HW exec, PE-bound (PE union busy ~139 us;
projections are DMA-overlapped; causal skipping + the balanced query split
keep all 8 cores within ~5% of each other).
"""

import numpy as np

B, T, E, D = 4, 4096, 2048, 128
NCORES = 8
G = 512                     # query-group / t-group width
NG = T // G                 # 8 t-groups
EC = E // 128               # 16 e-chunks
QSLOTS = [0, 3, 4, 7]       # t-group holding slot s's queries (canonical order)
NK = [1024, 2048, 3072, 4096]  # key-prefix length per slot
NMASK = 8                   # mask applied to the last 8 k-chunks of each slot
CHUNKS = [[0, 3, 4, 7], [1, 2, 5, 6]]    # global query chunk of (h, slot)
PERMS = [[0, 1, 2, 3, 4, 5, 6, 7], [1, 0, 3, 2, 5, 4, 7, 6]]  # key col perm per h
SCALE = 1.0 / float(np.sqrt(D))

_CACHE = {}


def _emit(nc, tc, ctx, aps):
    import concourse.bass as bass  # noqa: F401
    from concourse import mybir

    f32 = mybir.dt.float32
    f32r = mybir.dt.float32r
    xt, wq, wk, wv, thr, ident, out = (
        aps["xt"], aps["wq"], aps["wk"], aps["wv"], aps["thr"], aps["ident"],
        aps["out"],
    )

    # ---- pools ----
    wpool = ctx.enter_context(tc.tile_pool(name="w", bufs=1))
    cpool = ctx.enter_context(tc.tile_pool(name="const", bufs=1))
    xpool = ctx.enter_context(tc.tile_pool(name="xt", bufs=8))
    ktpool = ctx.enter_context(tc.tile_pool(name="kt", bufs=NG))
    vtpool = ctx.enter_context(tc.tile_pool(name="vt", bufs=2))
    vnpool = ctx.enter_context(tc.tile_pool(name="vn", bufs=NG))
    qtpool = ctx.enter_context(tc.tile_pool(name="qt", bufs=4))
    epool = ctx.enter_context(tc.tile_pool(name="e", bufs=6))
    dpool_sb = ctx.enter_context(tc.tile_pool(name="dsb", bufs=2))
    opool_sb = ctx.enter_context(tc.tile_pool(name="osb", bufs=2))

    ppool = ctx.enter_context(tc.tile_pool(name="pp", bufs=3, space="PSUM"))
    spool = ctx.enter_context(tc.tile_pool(name="sp", bufs=3, space="PSUM"))
    apool = ctx.enter_context(tc.tile_pool(name="av", bufs=1, space="PSUM"))
    denp = ctx.enter_context(tc.tile_pool(name="den", bufs=1, space="PSUM"))

    # ---- constants ----
    w_sb = {}
    for name, ap in (("wq", wq), ("wk", wk), ("wv", wv)):
        t = wpool.tile([128, EC, 128], f32r, tag=name, name=name)
        w_sb[name] = t
    for r in range(0, EC, 4):
        for name, ap in (("wk", wk), ("wv", wv), ("wq", wq)):
            nc.gpsimd.dma_start(
                out=w_sb[name][:, r:r + 4, :], in_=ap[:, r:r + 4, :])
    identity = cpool.tile([128, 128], f32, tag="ident", name="ident")
    nc.gpsimd.dma_start(out=identity[:, :], in_=ident[:, :])
    thr_sb = cpool.tile([128, 4, NMASK], f32, tag="thr", name="thr")
    nc.gpsimd.dma_start(out=thr_sb[:, :, :], in_=thr.rearrange("s i k -> k s i"))
    ones_f = cpool.tile([128, 1], f32, tag="ones_f", name="ones_f")
    nc.vector.memset(ones_f[:, :], 1.0)
    ones = cpool.tile([128, 1], f32r, tag="ones", name="ones")
    nc.vector.tensor_copy(out=ones[:, :], in_=ones_f[:, :])
    iota = cpool.tile([128, G], f32, tag="iota", name="iota")
    nc.gpsimd.iota(
        iota[:, :], pattern=[[1, G]], base=0, channel_multiplier=0,
        allow_small_or_imprecise_dtypes=True,
    )
    # Warm the PE HAM clock gate (~3.4us of busy flips K from 4/8 to 8/8)
    # with dummy matmuls while the first x/W DMAs are still streaming.
    wsc = cpool.tile([128, G], f32, tag="wsc", name="wsc")
    nc.vector.memset(wsc[:, :], 1.0)
    wps = spool.tile([128, G], f32, tag="sp", name="wps")
    for _ in range(3):
        nc.tensor.matmul(
            wps[:, :], wsc[:, 0:128], wsc[:, :], start=True, stop=True)

    KT = [None] * NG   # [d=128, G] per t-group
    VN = [None] * NG   # V natural [t_in=128, 4*128 d-cols]
    QT = [None] * 4    # [d=128, G] per slot

    def proj_group(g):
        is_q = g in QSLOTS
        s = QSLOTS.index(g) if is_q else -1
        pk = ppool.tile([128, G], f32, tag="pp", name="pp")
        pv = ppool.tile([128, G], f32, tag="pp", name="pp")
        pq = ppool.tile([128, G], f32, tag="pp", name="pp") if is_q else None
        xt_r = xt.rearrange("(c p) t -> p c t", p=128)
        for q4 in range(4):
            xtile = xpool.tile([128, 4, G], f32r, tag="xt", name="xt")
            nc.sync.dma_start(
                out=xtile[:, :, :],
                in_=xt_r[:, q4 * 4:(q4 + 1) * 4, g * G:(g + 1) * G],
            )
            for i in range(4):
                j = q4 * 4 + i
                rhs = xtile[:, i, :]
                st, sp = j == 0, j == EC - 1
                nc.tensor.matmul(
                    pk[:, :], w_sb["wk"][:, j, :], rhs, start=st, stop=sp)
                nc.tensor.matmul(
                    pv[:, :], w_sb["wv"][:, j, :], rhs, start=st, stop=sp)
                if is_q:
                    nc.tensor.matmul(
                        pq[:, :], w_sb["wq"][:, j, :], rhs,
                        start=st, stop=sp)
        kt = ktpool.tile([128, G], f32r, tag="kt", name="kt")
        nc.scalar.copy(out=kt[:, :], in_=pk[:, :])
        KT[g] = kt
        vt = vtpool.tile([128, G], f32, tag="vt", name="vt")
        nc.scalar.copy(out=vt[:, :], in_=pv[:, :])
        if is_q:
            qt = qtpool.tile([128, G], f32r, tag="qt", name="qt")
            nc.scalar.copy(out=qt[:, :], in_=pq[:, :])
            QT[s] = qt
        vn = vnpool.tile([128, G], f32r, tag="vn", name="vn")
        for c in range(4):
            pt = spool.tile([128, 128], f32, tag="sp", name="tp")
            nc.tensor.transpose(
                pt[:, :], vt[:, c * 128:(c + 1) * 128], identity[:, :])
            nc.vector.tensor_copy(
                out=vn[:, c * 128:(c + 1) * 128], in_=pt[:, :])
        VN[g] = vn

    def attn_slot(s):
        nchunks = NK[s] // 128
        po = apool.tile([128, G], f32, tag="av", name="av")
        pd = denp.tile([1, G], f32, tag="den", name="den")
        for c in range(nchunks):
            g, cc = c // 4, c % 4
            ps = spool.tile([128, G], f32, tag="sp", name="sp")
            nc.tensor.matmul(
                ps[:, :],
                KT[g][:, cc * 128:(cc + 1) * 128],
                QT[s][:, :],
                start=True, stop=True,
            )
            e = epool.tile([128, G], f32r, tag="e", name="e")
            nc.scalar.activation(
                out=e[:, :], in_=ps[:, :],
                func=mybir.ActivationFunctionType.Exp, scale=SCALE,
            )
            mi = c - (nchunks - NMASK)
            if mi >= 0:
                nc.vector.scalar_tensor_tensor(
                    out=e[:, :],
                    in0=iota[:, :],
                    scalar=thr_sb[:, s, mi:mi + 1],
                    in1=e[:, :],
                    op0=mybir.AluOpType.is_ge,
                    op1=mybir.AluOpType.mult,
                )
            nc.tensor.matmul(
                po[:, :], VN[g][:, cc * 128:(cc + 1) * 128], e[:, :],
                start=(c == 0), stop=(c == nchunks - 1))
            if c % 4 == 0:
                e_acc = e
            else:
                # accumulate exp tiles on DVE so the denominator needs one
                # PE matmul per 4 chunks
                esum = epool.tile([128, G], f32r, tag="es", name="es")
                nc.vector.tensor_add(esum[:, :], e_acc[:, :], e[:, :])
                e_acc = esum
                if c % 4 == 3:
                    nc.tensor.matmul(
                        pd[:, :], ones[:, :], esum[:, :],
                        start=(c == 3), stop=(c == nchunks - 1))
        dr = dpool_sb.tile([1, G], f32, tag="dr", name="dr")
        nc.vector.tensor_copy(out=dr[:, :], in_=pd[:, :])
        rr = dpool_sb.tile([1, G], f32, tag="rr", name="rr")
        rs = dpool_sb.tile([1, G], f32, tag="rs", name="rs")
        nc.vector.reciprocal_approx_accurate(out=rr[:, :], in_=dr[:, :], scratch=rs[:, :])
        db = dpool_sb.tile([128, G], f32, tag="db", name="db")
        nc.gpsimd.partition_broadcast(db[:, :], rr[:, :])
        osb = opool_sb.tile([128, G], f32, tag="osb", name="osb")
        nc.vector.tensor_mul(osb[:, :], po[:, :], db[:, :])
        nc.sync.dma_start(out=out[s, :, :], in_=osb[:, :])

    # Interleave projection groups and attention slots so the PE always has
    # attention work available while projection DMA streams.
    proj_group(0)
    proj_group(1)
    attn_slot(0)
    proj_group(2)
    proj_group(3)
    attn_slot(1)
    proj_group(4)
    proj_group(5)
    attn_slot(2)
    proj_group(6)
    proj_group(7)
    attn_slot(3)


def _build():
    if "nc" in _CACHE:
        return _CACHE["nc"]
    from contextlib import ExitStack

    import concourse.bacc as bacc
    import concourse.tile as tile
    from concourse import mybir

    f32 = mybir.dt.float32
    f32r = mybir.dt.float32r
    nc = bacc.Bacc(
        "TRN2", target_bir_lowering=False, debug=False, enable_asserts=False,
        num_devices=NCORES,
    )
    aps = {
        "xt": nc.dram_tensor("xt", [E, T], f32r, kind="ExternalInput").ap(),
        "wq": nc.dram_tensor("wq", [128, EC, D], f32r, kind="ExternalInput").ap(),
        "wk": nc.dram_tensor("wk", [128, EC, D], f32r, kind="ExternalInput").ap(),
        "wv": nc.dram_tensor("wv", [128, EC, D], f32r, kind="ExternalInput").ap(),
        "thr": nc.dram_tensor(
            "thr", [4, NMASK, 128], f32, kind="ExternalInput").ap(),
        "ident": nc.dram_tensor(
            "ident", [128, 128], f32, kind="ExternalInput").ap(),
        "out": nc.dram_tensor("out", [4, 128, G], f32, kind="ExternalOutput").ap(),
    }
    with tile.TileContext(nc) as tc, ExitStack() as ctx:
        _emit(nc, tc, ctx, aps)
    nc.compile()
    _CACHE["nc"] = nc
    return nc


def _thresholds(h):
    """thr[s, i, k_in]: min allowed local query col for key row k_in of the
    i-th masked k-chunk (chunk c = NK[s]/128 - NMASK + i) of slot s."""
    perm = np.asarray(PERMS[h])
    thr = np.zeros((4, NMASK, 128), dtype=np.float32)
    for s in range(4):
        qc = CHUNKS[h][s]
        c0 = NK[s] // 128 - NMASK
        for i in range(NMASK):
            pos = (c0 + i) * 128 + np.arange(128)        # permuted key column
            k_orig = perm[pos // G] * G + pos % G        # original key index
            thr[s, i] = np.clip(k_orig - qc * G, 0, G)
    return thr


def make_in_maps(x, Wq, Wk, Wv):
    x = np.ascontiguousarray(x, dtype=np.float32)
    def wshape(W):
        # [E, D] -> [128, EC, D]: chunk c rows c*128..c*128+127 at [:, c, :]
        return np.ascontiguousarray(
            np.asarray(W, dtype=np.float32).reshape(EC, 128, D).transpose(1, 0, 2))

    common = {
        "wq": wshape(Wq), "wk": wshape(Wk), "wv": wshape(Wv),
        "ident": np.eye(128, dtype=np.float32),
    }
    thrs = [_thresholds(0), _thresholds(1)]
    in_maps = []
    for c in range(NCORES):
        b, h = c // 2, c % 2
        xr = x[b].reshape(NG, G, E)[PERMS[h]]            # [8, 512, E] permuted
        xt = np.ascontiguousarray(
            xr.transpose(2, 0, 1).reshape(E, T))          # [E, T]
        in_maps.append({**common, "xt": xt, "thr": thrs[h]})
    return in_maps


def gather(results):
    out = np.empty((B, T, D), dtype=np.float32)
    for c in range(NCORES):
        b, h = c // 2, c % 2
        o = results[c]["out"]                             # [4, 128, 512]
        for s in range(4):
            qc = CHUNKS[h][s]
            out[b, qc * G:(qc + 1) * G, :] = o[s].T
    return out


def run(x, Wq, Wk, Wv, trace=False, **trace_kwargs):
    from concourse.bass_utils import run_bass_kernel_spmd

    nc = _build()
    in_maps = make_in_maps(x, Wq, Wk, Wv)
    res = run_bass_kernel_spmd(
        nc, in_maps, core_ids=list(range(NCORES)), trace=trace, **trace_kwargs)
    return gather(res.results), res


def kernel(x, Wq, Wk, Wv):
    out, _ = run(np.asarray(x), np.asarray(Wq), np.asarray(Wk), np.asarray(Wv))
    return out



# revision 7
# speedup vs baseline: 1.3574x; 1.3574x over previous
"""Single-head causal attention (prefill) on 8 Trainium2 NeuronCores.

Problem: x[4,4096,2048], Wq/Wk/Wv[2048,128] -> out[4,4096,128]
  Q=xWq K=xWk V=xWv; out = softmax(mask(QK^T/sqrt(128))) V

Sharding: data-parallel over batch (4) x flash-style 2-way KEY split.
Core c handles batch c//2; half h=c%2 OWNS the 256-wide key blocks with
(key//256)%2 == h (2048 keys).  Each core:
  - projects K,V only for its owned keys (the big dedup win vs. splitting
    queries: no duplicated K/V projection across the pair),
  - projects Q for all 4096 queries,
  - computes partial attention numerator num[g] = sum_k e(k,q) V[k] and
    partial denominator den[g] = sum_k e(k,q) over its OWNED keys only,
    for every 512-query group g (causal: group g sees 2(g+1) owned
    128-key chunks; the last 2 are the in-group diagonal, masked on-device
    via threshold data: e = (iota_q >= thr_k) * exp(score)).
Host combines: out = (num_even + num_odd) / (den_even + den_odd).

All matmul operands are bf16 (fp32 PSUM accumulate): same PE streaming
rate as fp32r but half the DMA traffic, half the DVE element cost, and
fast-weight-load halves LDWEIGHTS so it hides behind the matmuls.

On-device per core (all N=512-col matmuls unless noted):
  proj KV: 4 tiles x 16 e-chunks x {K,V}      = 128 MM
  proj Q:  8 tiles x 16 e-chunks              = 128 MM
  scores:  sum_g 2(g+1) chunk MMs             =  72 MM
  attnV:   same                               =  72 MM
  den:     ones[128,1]^T @ (4-chunk e sums)   =  20 MM (M=1)
  V transposes (PE, 128x128 bf16)             =  16
"""

import numpy as np

B, T, E, D = 4, 4096, 2048, 128
NCORES = 8
G = 512                     # query-group width (PSUM bank = 512 fp32)
NG = T // G                 # 8 query groups
EC = E // 128               # 16 e-chunks
OK = T // 2                 # 2048 owned keys per core
HB = 256                    # ownership half-block width
SCALE = 1.0 / float(np.sqrt(D))

_CACHE = {}


def _emit(nc, tc, ctx, aps):
    import concourse.bass as bass  # noqa: F401
    from concourse import mybir

    f32 = mybir.dt.float32
    bf16 = mybir.dt.bfloat16
    xq, xkv, wq, wk, wv, thr, ident, num, den = (
        aps["xq"], aps["xkv"], aps["wq"], aps["wk"], aps["wv"], aps["thr"],
        aps["ident"], aps["num"], aps["den"],
    )

    # ---- pools ----
    wpool = ctx.enter_context(tc.tile_pool(name="w", bufs=1))
    cpool = ctx.enter_context(tc.tile_pool(name="const", bufs=1))
    xpool = ctx.enter_context(tc.tile_pool(name="xt", bufs=20))
    kpool = ctx.enter_context(tc.tile_pool(name="kt", bufs=1))
    vtpool = ctx.enter_context(tc.tile_pool(name="vt", bufs=2))
    qtpool = ctx.enter_context(tc.tile_pool(name="qt", bufs=3))
    epool = ctx.enter_context(tc.tile_pool(name="e", bufs=6))
    espool = ctx.enter_context(tc.tile_pool(name="es", bufs=3))
    opool_sb = ctx.enter_context(tc.tile_pool(name="osb", bufs=2))
    dpool_sb = ctx.enter_context(tc.tile_pool(name="dsb", bufs=1))

    ppool = ctx.enter_context(tc.tile_pool(name="pp", bufs=2, space="PSUM"))
    spool = ctx.enter_context(tc.tile_pool(name="sp", bufs=3, space="PSUM"))
    apool = ctx.enter_context(tc.tile_pool(name="av", bufs=2, space="PSUM"))
    denp = ctx.enter_context(tc.tile_pool(name="den", bufs=1, space="PSUM"))

    # ---- constants (gpsimd DMA queue; x streams on sync, outputs on scalar)
    w_sb = {}
    for name, ap in (("wq", wq), ("wk", wk), ("wv", wv)):
        w_sb[name] = wpool.tile([128, EC, 128], bf16, tag=name, name=name)
    for r in range(0, EC, 4):
        for name, ap in (("wk", wk), ("wv", wv), ("wq", wq)):
            nc.gpsimd.dma_start(
                out=w_sb[name][:, r:r + 4, :], in_=ap[:, r:r + 4, :])
    identity = cpool.tile([128, 128], bf16, tag="ident", name="ident")
    nc.gpsimd.dma_start(out=identity[:, :], in_=ident[:, :])
    thr_sb = cpool.tile([128, 2], f32, tag="thr", name="thr")
    nc.gpsimd.dma_start(out=thr_sb[:, :], in_=thr[:, :])
    ones = cpool.tile([128, 1], bf16, tag="ones", name="ones")
    nc.vector.memset(ones[:, :], 1.0)
    iota = cpool.tile([128, G], f32, tag="iota", name="iota")
    nc.gpsimd.iota(
        iota[:, :], pattern=[[1, G]], base=0, channel_multiplier=0,
        allow_small_or_imprecise_dtypes=True,
    )
    # Warm the PE HAM clock gate while the first x/W DMAs stream.
    wsc = cpool.tile([128, G], bf16, tag="wsc", name="wsc")
    nc.vector.memset(wsc[:, :], 1.0)
    wps = spool.tile([128, G], f32, tag="sp", name="wps")
    for _ in range(6):
        nc.tensor.matmul(
            wps[:, :], wsc[:, 0:128], wsc[:, :], start=True, stop=True)

    KT = kpool.tile([128, OK], bf16, tag="KT", name="KT")
    VN = kpool.tile([128, OK], bf16, tag="VN", name="VN")
    QT = [None] * NG

    xq_r = xq.rearrange("(c p) t -> p c t", p=128)
    xkv_r = xkv.rearrange("(c p) t -> p c t", p=128)

    def proj_kv(j):
        """K,V projection of owned-key tile j (keys [512j, 512j+512))."""
        pk = ppool.tile([128, G], f32, tag="pp", name="pk")
        pv = ppool.tile([128, G], f32, tag="pp", name="pv")
        for q4 in range(4):
            xt = xpool.tile([128, 4, G], bf16, tag="xt", name="xkv")
            nc.sync.dma_start(
                out=xt[:, :, :],
                in_=xkv_r[:, q4 * 4:(q4 + 1) * 4, j * G:(j + 1) * G],
            )
            for i in range(4):
                jj = q4 * 4 + i
                rhs = xt[:, i, :]
                st, sp = jj == 0, jj == EC - 1
                nc.tensor.matmul(
                    pk[:, :], w_sb["wk"][:, jj, :], rhs, start=st, stop=sp)
                nc.tensor.matmul(
                    pv[:, :], w_sb["wv"][:, jj, :], rhs, start=st, stop=sp)
        nc.scalar.copy(out=KT[:, j * G:(j + 1) * G], in_=pk[:, :])
        vt = vtpool.tile([128, G], bf16, tag="vt", name="vt")
        nc.scalar.copy(out=vt[:, :], in_=pv[:, :])
        for c in range(4):
            pt = spool.tile([128, 128], bf16, tag="sp", name="tp")
            nc.tensor.transpose(
                pt[:, :], vt[:, c * 128:(c + 1) * 128], identity[:, :])
            nc.vector.tensor_copy(
                out=VN[:, (j * 4 + c) * 128:(j * 4 + c + 1) * 128],
                in_=pt[:, :])

    def proj_q(g):
        """Q projection of query group g (all queries)."""
        pq = ppool.tile([128, G], f32, tag="pp", name="pq")
        for q4 in range(4):
            xt = xpool.tile([128, 4, G], bf16, tag="xt", name="xq")
            nc.sync.dma_start(
                out=xt[:, :, :],
                in_=xq_r[:, q4 * 4:(q4 + 1) * 4, g * G:(g + 1) * G],
            )
            for i in range(4):
                jj = q4 * 4 + i
                st, sp = jj == 0, jj == EC - 1
                nc.tensor.matmul(
                    pq[:, :], w_sb["wq"][:, jj, :], xt[:, i, :],
                    start=st, stop=sp)
        qt = qtpool.tile([128, G], bf16, tag="qt", name="qt")
        nc.scalar.copy(out=qt[:, :], in_=pq[:, :])
        QT[g] = qt

    den_sb = dpool_sb.tile([1, NG * G], f32, tag="den", name="den_sb")

    def attn(g):
        """Partial attention of query group g over owned-key prefix."""
        nch = 2 * (g + 1)
        po = apool.tile([128, G], f32, tag="av", name="po")
        pd = denp.tile([1, G], f32, tag="den", name="pd")
        e_acc = None
        for c in range(nch):
            ps = spool.tile([128, G], f32, tag="sp", name="ps")
            nc.tensor.matmul(
                ps[:, :], KT[:, c * 128:(c + 1) * 128], QT[g][:, :],
                start=True, stop=True)
            e = epool.tile([128, G], bf16, tag="e", name="e")
            nc.scalar.activation(
                out=e[:, :], in_=ps[:, :],
                func=mybir.ActivationFunctionType.Exp, scale=SCALE,
            )
            mi = c - (nch - 2)
            if mi >= 0:
                nc.vector.scalar_tensor_tensor(
                    out=e[:, :],
                    in0=iota[:, :],
                    scalar=thr_sb[:, mi:mi + 1],
                    in1=e[:, :],
                    op0=mybir.AluOpType.is_ge,
                    op1=mybir.AluOpType.mult,
                )
            nc.tensor.matmul(
                po[:, :], VN[:, c * 128:(c + 1) * 128], e[:, :],
                start=(c == 0), stop=(c == nch - 1))
            # denominator: accumulate 4 e-chunks on DVE, then one PE matmul
            if c % 4 == 0:
                e_acc = e
            else:
                es = espool.tile([128, G], bf16, tag="es", name="es")
                nc.vector.tensor_add(es[:, :], e_acc[:, :], e[:, :])
                e_acc = es
            if c % 4 == 3 or c == nch - 1:
                nc.tensor.matmul(
                    pd[:, :], ones[:, :], e_acc[:, :],
                    start=(c < 4), stop=(c == nch - 1))
        osb = opool_sb.tile([128, G], f32, tag="osb", name="osb")
        nc.vector.tensor_copy(out=osb[:, :], in_=po[:, :])
        nc.scalar.dma_start(out=num[g, :, :], in_=osb[:, :])
        nc.vector.tensor_copy(
            out=den_sb[:, g * G:(g + 1) * G], in_=pd[:, :])

    # Schedule: stream projections (DMA-hungry) early, attention (DMA-free)
    # as soon as its KV prefix + Q group are resident.
    proj_kv(0)
    proj_q(0)
    attn(0)
    proj_q(1)
    attn(1)
    proj_kv(1)
    proj_q(2)
    attn(2)
    proj_q(3)
    attn(3)
    proj_kv(2)
    proj_q(4)
    attn(4)
    proj_q(5)
    attn(5)
    proj_kv(3)
    proj_q(6)
    attn(6)
    proj_q(7)
    attn(7)
    nc.scalar.dma_start(out=den[:, :], in_=den_sb[:, :])


def _build():
    if "nc" in _CACHE:
        return _CACHE["nc"]
    from contextlib import ExitStack

    import concourse.bacc as bacc
    import concourse.tile as tile
    from concourse import mybir

    f32 = mybir.dt.float32
    bf16 = mybir.dt.bfloat16
    nc = bacc.Bacc(
        "TRN2", target_bir_lowering=False, debug=False, enable_asserts=False,
        num_devices=NCORES,
    )
    aps = {
        "xq": nc.dram_tensor("xq", [E, T], bf16, kind="ExternalInput").ap(),
        "xkv": nc.dram_tensor("xkv", [E, OK], bf16, kind="ExternalInput").ap(),
        "wq": nc.dram_tensor("wq", [128, EC, D], bf16, kind="ExternalInput").ap(),
        "wk": nc.dram_tensor("wk", [128, EC, D], bf16, kind="ExternalInput").ap(),
        "wv": nc.dram_tensor("wv", [128, EC, D], bf16, kind="ExternalInput").ap(),
        "thr": nc.dram_tensor("thr", [128, 2], f32, kind="ExternalInput").ap(),
        "ident": nc.dram_tensor(
            "ident", [128, 128], bf16, kind="ExternalInput").ap(),
        "num": nc.dram_tensor(
            "num", [NG, 128, G], f32, kind="ExternalOutput").ap(),
        "den": nc.dram_tensor(
            "den", [1, NG * G], f32, kind="ExternalOutput").ap(),
    }
    with tile.TileContext(nc) as tc, ExitStack() as ctx:
        _emit(nc, tc, ctx, aps)
    nc.compile()
    _CACHE["nc"] = nc
    return nc


def make_in_maps(x, Wq, Wk, Wv):
    import ml_dtypes

    bf = ml_dtypes.bfloat16
    x = np.asarray(x, dtype=np.float32)

    def wshape(W):
        # [E, D] -> [128, EC, D]: chunk c rows c*128..c*128+127 at [:, c, :]
        return np.ascontiguousarray(
            np.asarray(W, dtype=np.float32).reshape(EC, 128, D)
            .transpose(1, 0, 2).astype(bf))

    common = {
        "wq": wshape(Wq), "wk": wshape(Wk), "wv": wshape(Wv),
        "ident": np.eye(128, dtype=np.float32).astype(bf),
    }
    row = np.arange(128, dtype=np.float32)
    thrs = [
        np.stack([256.0 * h + row, 256.0 * h + 128.0 + row], axis=1)
        .astype(np.float32)
        for h in range(2)
    ]
    idx = np.arange(T)
    sel = [idx[(idx // HB) % 2 == h] for h in range(2)]
    in_maps = []
    xq_b = {}
    for c in range(NCORES):
        b, h = c // 2, c % 2
        if b not in xq_b:
            xq_b[b] = np.ascontiguousarray(x[b].T.astype(bf))     # [E, T]
        xkv = np.ascontiguousarray(x[b][sel[h]].T.astype(bf))     # [E, OK]
        in_maps.append(
            {**common, "xq": xq_b[b], "xkv": xkv, "thr": thrs[h]})
    return in_maps


def gather(results):
    out = np.empty((B, T, D), dtype=np.float32)
    for b in range(B):
        rE, rO = results[2 * b], results[2 * b + 1]
        nsum = rE["num"] + rO["num"]                  # [NG, 128, G]
        dsum = (rE["den"] + rO["den"]).reshape(NG, 1, G)
        out[b] = (nsum / dsum).transpose(0, 2, 1).reshape(T, D)
    return out


def run(x, Wq, Wk, Wv, trace=False, **trace_kwargs):
    from concourse.bass_utils import run_bass_kernel_spmd

    nc = _build()
    in_maps = make_in_maps(x, Wq, Wk, Wv)
    res = run_bass_kernel_spmd(
        nc, in_maps, core_ids=list(range(NCORES)), trace=trace, **trace_kwargs)
    return gather(res.results), res


def kernel(x, Wq, Wk, Wv):
    out, _ = run(np.asarray(x), np.asarray(Wq), np.asarray(Wk), np.asarray(Wv))
    return out
